# revision 2
# baseline (speedup 1.0000x reference)
"""CrossNonLocal2D kernel v2 for Trainium2, 8-way batch-parallel SPMD.

This environment charges a ~85-90us fixed cost per instruction regardless of
operand size, with no cross-engine overlap, so the design minimizes total
instruction count rather than engine occupancy:

Per core (one batch element b):
  theta = theta_w @ xt + tb      [I, N]   (fp32 matmuls, no input casts)
  phi   = phi_w   @ xo + pb      [I, N]
  gT    = (g_w @ xo)^T           [N, I]   (computed directly transposed,
                                           4 m-tiles share one PSUM bank)
  ST    = phi^T @ theta          [m, n]   (attention logits, transposed)
  PT    = exp(ST)                one ACT instruction per 4 PSUM banks
  yuT   = g @ P^T                [I, n]   directly via lhsT=gT tiles: 32
                                          accumulating matmuls per 512-chunk
                                          (vs 1024 total in the y-orientation)
  d     = colsum(PT)             1 strided DVE reduce + 1 ones-matmul per chunk
  yT    = yuT * (1/d)            reciprocal + partition_broadcast + 1 multiply
  out   = x_this + w_eff @ yT + b_eff   (BN + g/out biases folded on host)
"""

import os
import sys
import time

import numpy as np

for _p in ("/opt/trn_rl_repo",):
    if os.path.isdir(_p) and _p not in sys.path:
        sys.path.insert(0, _p)

import ml_dtypes  # noqa: E402
import concourse.bacc as bacc  # noqa: E402
import concourse.mybir as mybir  # noqa: E402
import concourse.tile as tile  # noqa: E402
from concourse.bass import ts  # noqa: E402
from concourse.bass_utils import run_bass_kernel_spmd  # noqa: E402

B, C, HH, WW = 8, 256, 64, 64
N = HH * WW  # 4096
I = 128  # inter channels
NCORES = 8
BN_EPS = 1e-5
NCH = N // 512  # 8 n-chunks of 512
MT = N // 128  # 32 m-tiles of 128

f32 = mybir.dt.float32
bf16 = mybir.dt.bfloat16
EXP = mybir.ActivationFunctionType.Exp
ADD = mybir.AluOpType.add
MULT = mybir.AluOpType.mult
AXX = mybir.AxisListType.X


def build_module(repeat: int = 1):
    nc = bacc.Bacc("TRN2", target_bir_lowering=False, debug=False,
                   num_devices=NCORES)

    xt_d = nc.dram_tensor("xt", [C, N], f32, kind="ExternalInput")
    xo_d = nc.dram_tensor("xo", [C, N], f32, kind="ExternalInput")
    thwT_d = nc.dram_tensor("thwT", [C, I], f32, kind="ExternalInput")
    phwT_d = nc.dram_tensor("phwT", [C, I], f32, kind="ExternalInput")
    gwT_d = nc.dram_tensor("gwT", [C, I], f32, kind="ExternalInput")
    weffT_d = nc.dram_tensor("weffT", [I, C], bf16, kind="ExternalInput")
    tb_d = nc.dram_tensor("tb", [I, 1], f32, kind="ExternalInput")
    pb_d = nc.dram_tensor("pb", [I, 1], f32, kind="ExternalInput")
    beff_d = nc.dram_tensor("beff", [128, 2], f32, kind="ExternalInput")
    out_d = nc.dram_tensor("out", [C, N], f32, kind="ExternalOutput")

    # DRAM views with the c dim split as c = a*128 + p  (p = partition)
    xt_v = xt_d.ap().rearrange("(a p) n -> p a n", p=128)
    xo_v = xo_d.ap().rearrange("(a p) n -> p a n", p=128)
    out_v = out_d.ap().rearrange("(a p) n -> p a n", p=128)

    with tile.TileContext(nc) as tc:
        with (
            tc.tile_pool(name="const", bufs=1) as constp,
            tc.tile_pool(name="xbig", bufs=1) as xbig,
            tc.tile_pool(name="chunk", bufs=1) as chp,
            tc.tile_pool(name="ptp", bufs=2) as ptp,
            tc.tile_pool(name="small", bufs=2) as smp,
            tc.tile_pool(name="outp", bufs=2) as outp,
            tc.tile_pool(name="pst", bufs=1, space="PSUM") as ps_st,
            tc.tile_pool(name="poc", bufs=2, space="PSUM") as ps_oc,
            tc.tile_pool(name="pgd", bufs=1, space="PSUM") as ps_gd,
            tc.tile_pool(name="pyu", bufs=1, space="PSUM") as ps_yu,
        ):
            # ---- weights / constants (loaded once) ----
            thwT = constp.tile([128, 2, I], f32, tag="thwT")
            nc.sync.dma_start(out=thwT,
                              in_=thwT_d.ap().rearrange("(a p) i -> p a i", p=128))
            phwT = constp.tile([128, 2, I], f32, tag="phwT")
            nc.sync.dma_start(out=phwT,
                              in_=phwT_d.ap().rearrange("(a p) i -> p a i", p=128))
            gwT = constp.tile([128, 2, I], f32, tag="gwT")
            nc.sync.dma_start(out=gwT,
                              in_=gwT_d.ap().rearrange("(a p) i -> p a i", p=128))
            weffT = constp.tile([128, 2, 128], bf16, tag="weffT")
            nc.sync.dma_start(out=weffT,
                              in_=weffT_d.ap().rearrange("i (h c) -> i h c", h=2))
            tb = constp.tile([128, 1], f32, tag="tb")
            nc.sync.dma_start(out=tb, in_=tb_d.ap())
            pb = constp.tile([128, 1], f32, tag="pb")
            nc.sync.dma_start(out=pb, in_=pb_d.ap())
            beff = constp.tile([128, 2], f32, tag="beff")
            nc.sync.dma_start(out=beff, in_=beff_d.ap())
            ones = constp.tile([128, 1], f32, tag="ones")
            nc.gpsimd.memset(ones[:], 1.0)

            for _rep in range(repeat):
                xt_all = xbig.tile([128, 2, N], f32, tag="xt", name="xt_all")
                nc.sync.dma_start(out=xt_all, in_=xt_v)
                xo_all = xbig.tile([128, 2, N], f32, tag="xo", name="xo_all")
                nc.sync.dma_start(out=xo_all, in_=xo_v)

                th_c = chp.tile([128, NCH, 512], bf16, tag="th", name="th")
                ph_c = chp.tile([128, NCH, 512], bf16, tag="ph", name="ph")
                gTo = chp.tile([128, MT, 128], bf16, tag="gT", name="gT")

                # ---- 1x1 convs in fp32 straight from the loaded x ----
                for j in range(NCH):
                    ps_t = ps_oc.tile([128, 512], f32, tag="oc")
                    for a in range(2):
                        nc.tensor.matmul(ps_t[:],
                                         lhsT=thwT[:, a, :],
                                         rhs=xt_all[:, a, ts(j, 512)],
                                         start=(a == 0), stop=(a == 1))
                    nc.vector.tensor_scalar_add(th_c[:, j, :], ps_t[:], tb[:])
                    ps_p = ps_oc.tile([128, 512], f32, tag="oc")
                    for a in range(2):
                        nc.tensor.matmul(ps_p[:],
                                         lhsT=phwT[:, a, :],
                                         rhs=xo_all[:, a, ts(j, 512)],
                                         start=(a == 0), stop=(a == 1))
                    nc.vector.tensor_scalar_add(ph_c[:, j, :], ps_p[:], pb[:])
                    pg = ps_gd.tile([128, 4, 128], f32, tag="gd")
                    for k in range(4):
                        for a in range(2):
                            nc.tensor.matmul(
                                pg[:, k, :],
                                lhsT=xo_all[:, a, j * 512 + k * 128:
                                            j * 512 + (k + 1) * 128],
                                rhs=gwT[:, a, :],
                                start=(a == 0), stop=(a == 1))
                    nc.vector.tensor_copy(gTo[:, 4 * j:4 * j + 4, :], pg[:])

                # ---- attention: ST/exp of chunk jj interleaved with the
                # PV + epilogue of chunk jj-1 (PT window = 2 chunks) ----
                PT = [None] * NCH
                for jj in range(NCH + 1):
                    if jj < NCH:
                        PT[jj] = ptp.tile([128, MT, 512], bf16, tag="PT",
                                          name=f"PT{jj}")
                        for g in range(8):
                            pss = ps_st.tile([128, 4, 512], f32, tag="st")
                            for q in range(4):
                                nc.tensor.matmul(
                                    pss[:, q, :],
                                    lhsT=ph_c[:, g, ts(q, 128)],
                                    rhs=th_c[:, jj, :],
                                    start=True, stop=True)
                            nc.scalar.activation(
                                PT[jj][:, 4 * g:4 * g + 4, :], pss[:], EXP)
                    if jj >= 1:
                        j = jj - 1
                        PTp = PT[j]
                        pyu = ps_yu.tile([128, 512], f32, tag="yu")
                        for t in range(MT):
                            nc.tensor.matmul(pyu[:],
                                             lhsT=gTo[:, t, :],
                                             rhs=PTp[:, t, :],
                                             start=(t == 0),
                                             stop=(t == MT - 1))
                        tred = smp.tile([128, 512], f32, tag="tred")
                        nc.vector.tensor_reduce(
                            tred[:], PTp[:].rearrange("p t n -> p n t"),
                            axis=AXX, op=ADD)
                        pd = ps_gd.tile([1, 512], f32, tag="gd")
                        nc.tensor.matmul(pd[:], lhsT=ones[:], rhs=tred[:],
                                         start=True, stop=True)
                        rcp = smp.tile([1, 512], f32, tag="rcp")
                        nc.vector.reciprocal(rcp[:], pd[:])
                        rcpb = smp.tile([128, 512], f32, tag="rcpb")
                        nc.gpsimd.partition_broadcast(rcpb[:], rcp[:])
                        yT = smp.tile([128, 512], bf16, tag="yT")
                        nc.vector.tensor_tensor(yT[:], pyu[:], rcpb[:], op=MULT)
                        ob = outp.tile([128, 2, 512], f32, tag="ob")
                        for h in range(2):
                            oc = ps_oc.tile([128, 512], f32, tag="oc")
                            nc.tensor.matmul(oc[:], lhsT=weffT[:, h, :],
                                             rhs=yT[:], start=True, stop=True)
                            nc.vector.scalar_tensor_tensor(
                                ob[:, h, :], oc[:], beff[:, h:h + 1],
                                xt_all[:, h, ts(j, 512)], op0=ADD, op1=ADD)
                        nc.sync.dma_start(out=out_v[:, :, ts(j, 512)], in_=ob[:])

    nc.compile()
    return nc


_CACHE: dict = {}


def _get_built(repeat: int = 1):
    if repeat not in _CACHE:
        _CACHE[repeat] = build_module(repeat)
    return _CACHE[repeat]


def prep_maps(inputs: dict) -> list[dict]:
    """Host-side precompute: fold BN + g/out biases, transpose weights."""
    f = lambda k: np.asarray(inputs[k], np.float32)
    x_this = f("x_this").reshape(B, C, N)
    x_other = f("x_other").reshape(B, C, N)
    theta_w, theta_b = f("theta_w"), f("theta_b")
    phi_w, phi_b = f("phi_w"), f("phi_b")
    g_w, g_b = f("g_w"), f("g_b")
    out_w, out_b = f("out_w"), f("out_b")
    gam, bet = f("bn_gamma"), f("bn_beta")
    mean, var = f("bn_mean"), f("bn_var")

    s = (gam / np.sqrt(var + BN_EPS)).astype(np.float32)  # [C]
    w_eff = (out_w * s[:, None]).astype(np.float32)  # [C, I]
    b_eff = (s * (out_w @ g_b + out_b - mean) + bet).astype(np.float32)  # [C]

    bf = ml_dtypes.bfloat16
    common = {
        "thwT": np.ascontiguousarray(theta_w.T).astype(np.float32),
        "phwT": np.ascontiguousarray(phi_w.T).astype(np.float32),
        "gwT": np.ascontiguousarray(g_w.T).astype(np.float32),
        "weffT": np.ascontiguousarray(w_eff.T).astype(bf),
        "tb": np.ascontiguousarray(theta_b[:, None]),
        "pb": np.ascontiguousarray(phi_b[:, None]),
        "beff": np.ascontiguousarray(b_eff.reshape(2, 128).T),
    }
    return [
        {"xt": np.ascontiguousarray(x_this[b]),
         "xo": np.ascontiguousarray(x_other[b]), **common}
        for b in range(B)
    ]


def run(inputs: dict, repeat: int = 1, time_it: bool = False):
    nc = _get_built(repeat)
    maps = prep_maps(inputs)
    t0 = time.time()
    res = run_bass_kernel_spmd(nc, maps, list(range(NCORES)))
    wall = time.time() - t0
    out = np.stack([np.asarray(res.results[b]["out"], np.float32)
                    for b in range(B)])
    out = out.reshape(B, C, HH, WW)
    if time_it:
        return out, wall
    return out


def kernel(**inputs) -> np.ndarray:
    return run(inputs)


# revision 4
# speedup vs baseline: 6.9277x; 6.9277x over previous
"""CrossNonLocal2D kernel v2 for Trainium2, 8-way batch-parallel SPMD.

This environment charges a ~85-90us fixed cost per instruction regardless of
operand size, with no cross-engine overlap, so the design minimizes total
instruction count rather than engine occupancy:

Per core (one batch element b):
  theta = theta_w @ xt + tb      [I, N]   (fp32 matmuls, no input casts)
  phi   = phi_w   @ xo + pb      [I, N]
  gT    = (g_w @ xo)^T           [N, I]   (computed directly transposed,
                                           4 m-tiles share one PSUM bank)
  ST    = phi^T @ theta          [m, n]   (attention logits, transposed)
  PT    = exp(ST)                one ACT instruction per 4 PSUM banks
  yuT   = g @ P^T                [I, n]   directly via lhsT=gT tiles: 32
                                          accumulating matmuls per 512-chunk
                                          (vs 1024 total in the y-orientation)
  d     = colsum(PT)             1 strided DVE reduce + 1 ones-matmul per chunk
  yT    = yuT * (1/d)            reciprocal + partition_broadcast + 1 multiply
  out   = x_this + w_eff @ yT + b_eff   (BN + g/out biases folded on host)
"""

import os
import sys
import time

import numpy as np

for _p in ("/opt/trn_rl_repo",):
    if os.path.isdir(_p) and _p not in sys.path:
        sys.path.insert(0, _p)

import ml_dtypes  # noqa: E402
import concourse.bacc as bacc  # noqa: E402
import concourse.mybir as mybir  # noqa: E402
import concourse.tile as tile  # noqa: E402
from concourse.bass import ts  # noqa: E402
from concourse.bass_utils import run_bass_kernel_spmd  # noqa: E402

B, C, HH, WW = 8, 256, 64, 64
N = HH * WW  # 4096
I = 128  # inter channels
NCORES = 8
BN_EPS = 1e-5
NCH = N // 512  # 8 n-chunks of 512
MT = N // 128  # 32 m-tiles of 128

f32 = mybir.dt.float32
bf16 = mybir.dt.bfloat16
EXP = mybir.ActivationFunctionType.Exp
ADD = mybir.AluOpType.add
MULT = mybir.AluOpType.mult
AXX = mybir.AxisListType.X


def build_module(repeat: int = 1):
    nc = bacc.Bacc("TRN2", target_bir_lowering=False, debug=False,
                   num_devices=NCORES)

    xt_d = nc.dram_tensor("xt", [C, N], f32, kind="ExternalInput")
    xo_d = nc.dram_tensor("xo", [C, N], f32, kind="ExternalInput")
    thwT_d = nc.dram_tensor("thwT", [C, I], f32, kind="ExternalInput")
    phwT_d = nc.dram_tensor("phwT", [C, I], f32, kind="ExternalInput")
    gwT_d = nc.dram_tensor("gwT", [C, I], f32, kind="ExternalInput")
    weffT_d = nc.dram_tensor("weffT", [I, C], bf16, kind="ExternalInput")
    tb_d = nc.dram_tensor("tb", [I, 1], f32, kind="ExternalInput")
    pb_d = nc.dram_tensor("pb", [I, 1], f32, kind="ExternalInput")
    beff_d = nc.dram_tensor("beff", [128, 2], f32, kind="ExternalInput")
    out_d = nc.dram_tensor("out", [C, N], f32, kind="ExternalOutput")

    # DRAM views with the c dim split as c = a*128 + p  (p = partition)
    xt_v = xt_d.ap().rearrange("(a p) n -> p a n", p=128)
    xo_v = xo_d.ap().rearrange("(a p) n -> p a n", p=128)
    out_v = out_d.ap().rearrange("(a p) n -> p a n", p=128)

    with tile.TileContext(nc) as tc:
        with (
            tc.tile_pool(name="const", bufs=1) as constp,
            tc.tile_pool(name="xbig", bufs=1) as xbig,
            tc.tile_pool(name="chunk", bufs=1) as chp,
            tc.tile_pool(name="ptp", bufs=2) as ptp,
            tc.tile_pool(name="small", bufs=2) as smp,
            tc.tile_pool(name="outp", bufs=2) as outp,
            tc.tile_pool(name="pst", bufs=1, space="PSUM") as ps_st,
            tc.tile_pool(name="poc", bufs=2, space="PSUM") as ps_oc,
            tc.tile_pool(name="pgd", bufs=1, space="PSUM") as ps_gd,
            tc.tile_pool(name="pyu", bufs=1, space="PSUM") as ps_yu,
        ):
            # ---- weights / constants (loaded once) ----
            thwT = constp.tile([128, 2, I], f32, tag="thwT")
            nc.sync.dma_start(out=thwT,
                              in_=thwT_d.ap().rearrange("(a p) i -> p a i", p=128))
            phwT = constp.tile([128, 2, I], f32, tag="phwT")
            nc.sync.dma_start(out=phwT,
                              in_=phwT_d.ap().rearrange("(a p) i -> p a i", p=128))
            gwT = constp.tile([128, 2, I], f32, tag="gwT")
            nc.sync.dma_start(out=gwT,
                              in_=gwT_d.ap().rearrange("(a p) i -> p a i", p=128))
            weffT = constp.tile([128, 2, 128], bf16, tag="weffT")
            nc.sync.dma_start(out=weffT,
                              in_=weffT_d.ap().rearrange("i (h c) -> i h c", h=2))
            tb = constp.tile([128, 1], f32, tag="tb")
            nc.sync.dma_start(out=tb, in_=tb_d.ap())
            pb = constp.tile([128, 1], f32, tag="pb")
            nc.sync.dma_start(out=pb, in_=pb_d.ap())
            beff = constp.tile([128, 2], f32, tag="beff")
            nc.sync.dma_start(out=beff, in_=beff_d.ap())
            ones = constp.tile([128, 1], f32, tag="ones")
            nc.gpsimd.memset(ones[:], 1.0)

            for _rep in range(repeat):
                xt_all = xbig.tile([128, 2, N], f32, tag="xt", name="xt_all")
                nc.sync.dma_start(out=xt_all, in_=xt_v)
                xo_all = xbig.tile([128, 2, N], f32, tag="xo", name="xo_all")
                nc.sync.dma_start(out=xo_all, in_=xo_v)

                th_c = chp.tile([128, NCH, 512], bf16, tag="th", name="th")
                ph_c = chp.tile([128, NCH, 512], bf16, tag="ph", name="ph")
                gTo = chp.tile([128, MT, 128], bf16, tag="gT", name="gT")

                # ---- 1x1 convs in fp32 straight from the loaded x ----
                for j in range(NCH):
                    ps_t = ps_oc.tile([128, 512], f32, tag="oc")
                    for a in range(2):
                        nc.tensor.matmul(ps_t[:],
                                         lhsT=thwT[:, a, :],
                                         rhs=xt_all[:, a, ts(j, 512)],
                                         start=(a == 0), stop=(a == 1))
                    nc.vector.tensor_scalar_add(th_c[:, j, :], ps_t[:], tb[:])
                    ps_p = ps_oc.tile([128, 512], f32, tag="oc")
                    for a in range(2):
                        nc.tensor.matmul(ps_p[:],
                                         lhsT=phwT[:, a, :],
                                         rhs=xo_all[:, a, ts(j, 512)],
                                         start=(a == 0), stop=(a == 1))
                    nc.vector.tensor_scalar_add(ph_c[:, j, :], ps_p[:], pb[:])
                    pg = ps_gd.tile([128, 4, 128], f32, tag="gd")
                    for k in range(4):
                        for a in range(2):
                            nc.tensor.matmul(
                                pg[:, k, :],
                                lhsT=xo_all[:, a, j * 512 + k * 128:
                                            j * 512 + (k + 1) * 128],
                                rhs=gwT[:, a, :],
                                start=(a == 0), stop=(a == 1))
                    nc.vector.tensor_copy(gTo[:, 4 * j:4 * j + 4, :], pg[:])

                # ---- attention: ST/exp of chunk jj interleaved with the
                # PV + epilogue of chunk jj-1 (PT window = 2 chunks) ----
                PT = [None] * NCH
                for jj in range(NCH + 1):
                    if jj < NCH:
                        PT[jj] = ptp.tile([128, MT, 512], bf16, tag="PT",
                                          name=f"PT{jj}")
                        for g in range(8):
                            pss = ps_st.tile([128, 4, 512], f32, tag="st")
                            for q in range(4):
                                nc.tensor.matmul(
                                    pss[:, q, :],
                                    lhsT=ph_c[:, g, ts(q, 128)],
                                    rhs=th_c[:, jj, :],
                                    start=True, stop=True)
                            nc.scalar.activation(
                                PT[jj][:, 4 * g:4 * g + 4, :], pss[:], EXP)
                    if jj >= 1:
                        j = jj - 1
                        PTp = PT[j]
                        pyu = ps_yu.tile([128, 512], f32, tag="yu")
                        for t in range(MT):
                            nc.tensor.matmul(pyu[:],
                                             lhsT=gTo[:, t, :],
                                             rhs=PTp[:, t, :],
                                             start=(t == 0),
                                             stop=(t == MT - 1))
                        tred = smp.tile([128, 512], f32, tag="tred")
                        nc.vector.tensor_reduce(
                            tred[:], PTp[:].rearrange("p t n -> p n t"),
                            axis=AXX, op=ADD)
                        pd = ps_gd.tile([1, 512], f32, tag="gd")
                        nc.tensor.matmul(pd[:], lhsT=ones[:], rhs=tred[:],
                                         start=True, stop=True)
                        rcp = smp.tile([1, 512], f32, tag="rcp")
                        nc.vector.reciprocal(rcp[:], pd[:])
                        rcpb = smp.tile([128, 512], f32, tag="rcpb")
                        nc.gpsimd.partition_broadcast(rcpb[:], rcp[:])
                        yT = smp.tile([128, 512], bf16, tag="yT")
                        nc.vector.tensor_tensor(yT[:], pyu[:], rcpb[:], op=MULT)
                        ob = outp.tile([128, 2, 512], f32, tag="ob")
                        for h in range(2):
                            oc = ps_oc.tile([128, 512], f32, tag="oc")
                            nc.tensor.matmul(oc[:], lhsT=weffT[:, h, :],
                                             rhs=yT[:], start=True, stop=True)
                            nc.vector.scalar_tensor_tensor(
                                ob[:, h, :], oc[:], beff[:, h:h + 1],
                                xt_all[:, h, ts(j, 512)], op0=ADD, op1=ADD)
                        nc.sync.dma_start(out=out_v[:, :, ts(j, 512)], in_=ob[:])

    nc.compile()
    return nc


_CACHE: dict = {}


def _get_built(repeat: int = 1):
    if repeat not in _CACHE:
        _CACHE[repeat] = build_module(repeat)
    return _CACHE[repeat]


def _make_runner(nc, n_cores: int):
    """Cached sharded-jit runner: traces/lowers/compiles the module once and
    reuses the PJRT executable across calls (run_bass_kernel_spmd re-traces
    the whole module per call, which dominates wall time for big modules)."""
    import jax
    from jax.experimental.shard_map import shard_map
    from jax.sharding import Mesh, PartitionSpec
    from concourse import bass2jax

    bass2jax.install_neuronx_cc_hook()
    partition_name = (nc.partition_id_tensor.name
                      if nc.partition_id_tensor else None)
    in_names, out_names, out_avals, zero_shapes = [], [], [], []
    for alloc in nc.m.functions[0].allocations:
        if not isinstance(alloc, mybir.MemoryLocationSet):
            continue
        name = alloc.memorylocations[0].name
        if alloc.kind == "ExternalInput":
            if name != partition_name:
                in_names.append(name)
        elif alloc.kind == "ExternalOutput":
            out_names.append(name)
            shape = tuple(alloc.tensor_shape)
            dtype = mybir.dt.np(alloc.dtype)
            out_avals.append(jax.core.ShapedArray(shape, dtype))
            zero_shapes.append((shape, dtype))
    n_params = len(in_names)
    n_outs = len(out_avals)
    bind_in_names = list(in_names) + list(out_names)
    if partition_name is not None:
        bind_in_names.append(partition_name)
    donate = tuple(range(n_params, n_params + n_outs))

    def _body(*args):
        operands = list(args)
        if partition_name is not None:
            operands.append(bass2jax.partition_id_tensor())
        outs = bass2jax._bass_exec_p.bind(
            *operands,
            out_avals=tuple(out_avals),
            in_names=tuple(bind_in_names),
            out_names=tuple(out_names),
            lowering_input_output_aliases=(),
            sim_require_finite=True,
            sim_require_nnan=True,
            nc=nc,
        )
        return tuple(outs)

    devices = jax.devices()[:n_cores]
    mesh = Mesh(np.asarray(devices), ("core",))
    in_specs = (PartitionSpec("core"),) * (n_params + n_outs)
    out_specs = (PartitionSpec("core"),) * len(out_names)
    sharded = jax.jit(
        shard_map(_body, mesh=mesh, in_specs=in_specs, out_specs=out_specs,
                  check_rep=False),
        donate_argnums=donate, keep_unused=True)

    def run_maps(in_maps):
        per_core = [[np.asarray(m[nm]) for nm in in_names] for m in in_maps]
        concat_in = [
            np.concatenate([per_core[c][i] for c in range(n_cores)], axis=0)
            for i in range(n_params)
        ]
        concat_zeros = [np.zeros((n_cores * s[0], *s[1:]), d)
                        for (s, d) in zero_shapes]
        out_arrs = sharded(*concat_in, *concat_zeros)
        return [
            {nm: np.asarray(out_arrs[i]).reshape(n_cores, *out_avals[i].shape)[c]
             for i, nm in enumerate(out_names)}
            for c in range(n_cores)
        ]

    return run_maps


_RUNNERS: dict = {}


def _get_runner(repeat: int = 1):
    if repeat not in _RUNNERS:
        _RUNNERS[repeat] = _make_runner(_get_built(repeat), NCORES)
    return _RUNNERS[repeat]


def prep_maps(inputs: dict) -> list[dict]:
    """Host-side precompute: fold BN + g/out biases, transpose weights."""
    f = lambda k: np.asarray(inputs[k], np.float32)
    x_this = f("x_this").reshape(B, C, N)
    x_other = f("x_other").reshape(B, C, N)
    theta_w, theta_b = f("theta_w"), f("theta_b")
    phi_w, phi_b = f("phi_w"), f("phi_b")
    g_w, g_b = f("g_w"), f("g_b")
    out_w, out_b = f("out_w"), f("out_b")
    gam, bet = f("bn_gamma"), f("bn_beta")
    mean, var = f("bn_mean"), f("bn_var")

    s = (gam / np.sqrt(var + BN_EPS)).astype(np.float32)  # [C]
    w_eff = (out_w * s[:, None]).astype(np.float32)  # [C, I]
    b_eff = (s * (out_w @ g_b + out_b - mean) + bet).astype(np.float32)  # [C]

    bf = ml_dtypes.bfloat16
    common = {
        "thwT": np.ascontiguousarray(theta_w.T).astype(np.float32),
        "phwT": np.ascontiguousarray(phi_w.T).astype(np.float32),
        "gwT": np.ascontiguousarray(g_w.T).astype(np.float32),
        "weffT": np.ascontiguousarray(w_eff.T).astype(bf),
        "tb": np.ascontiguousarray(theta_b[:, None]),
        "pb": np.ascontiguousarray(phi_b[:, None]),
        "beff": np.ascontiguousarray(b_eff.reshape(2, 128).T),
    }
    return [
        {"xt": np.ascontiguousarray(x_this[b]),
         "xo": np.ascontiguousarray(x_other[b]), **common}
        for b in range(B)
    ]


def run(inputs: dict, repeat: int = 1, time_it: bool = False):
    maps = prep_maps(inputs)
    try:
        runner = _get_runner(repeat)
        t0 = time.time()
        results = runner(maps)
        wall = time.time() - t0
    except Exception:
        nc = _get_built(repeat)
        t0 = time.time()
        results = run_bass_kernel_spmd(nc, maps, list(range(NCORES))).results
        wall = time.time() - t0
    out = np.stack([np.asarray(results[b]["out"], np.float32)
                    for b in range(B)])
    out = out.reshape(B, C, HH, WW)
    if time_it:
        return out, wall
    return out


def kernel(**inputs) -> np.ndarray:
    return run(inputs)
